# revision 22
# baseline (speedup 1.0000x reference)
"""Trainium2 Bass kernel for nn_Attention_86165633892896 (sparse_attention).

Math: the reference scatters fresh k/v rows into a paged KV cache at
collision-free slots, then immediately gathers the same slots back out.
With unique slots, gather(scatter(cache, s, x), s) == x exactly, so the
cache round-trip is an identity and the output depends only on q, k, v:

    out[b] = softmax(Q_b @ K_b^T * scale) @ V_b        (per batch b)

with Q_b, K_b, V_b of shape [32, 128]  (32 heads, head_dim 128), B = 4096.

Scores are bounded (|s| < ~6 for randn inputs), so softmax without
max-subtraction is numerically safe in fp32 and matches jax.nn.softmax to
fp32 rounding.

Mapping to one NeuronCore (data-parallel over B, 512 batches/core):
  * batches are processed in "groups" of 4 -> a [128, 128] tile whose
    partition axis is (b_local*32 + head) and free axis is head_dim d.
  * Q,K group tiles are PE-transposed (fp32 transpose mode) to put d on
    partitions.
  * QK^T: 4 column-tiled matmuls (tile_position=(0,32j)), one per batch,
    stationary = Q^T[:, 32j:32j+32], moving = K^T[:, 32j:32j+32].
    Output lands compactly as PSUM [128=(4b,h), 32=k].
  * softmax: one ACT exp (scale folded in), one DVE reduce_sum, one DVE
    reciprocal.  1/denominator is folded into the output copy.
  * P^T: one DVE StreamTranspose (in-place 32x32 block transposes).
  * PV: 4 diagonal-tiled matmuls (tile_position=(32j,32j)), stationary =
    P_j^T [32k, 32h], moving = natural V rows [32k, 128d].  Output is the
    natural output layout [128=(4b,h), 128=d] in PSUM.
  * output: one DVE tensor_tensor multiply by broadcast reciprocal,
    PSUM -> SBUF, then contiguous DMA out.
Four groups form a "supergroup" sharing single softmax/copy instructions;
chunks of 16 groups share 1 MiB DMAs.
"""

import numpy as np

B = 4096
H = 32
D = 128
SCALE = 0.08838834764831845
NCORES = 8
NB = B // NCORES  # 512 batches per core

SUP = 4  # groups per supergroup (16 batches)


def build_kernel(nb=NB, gpc=16, loop_T=1, ablate=()):
    """Build the per-core Bass kernel for nb batches, gpc groups per DMA chunk.

    loop_T > 1 wraps the whole body in a For_i that repeats it (identical
    work each iteration) -- used only for device-time measurement.
    """
    import contextlib

    import concourse.bacc as bacc
    import concourse.mybir as mybir
    import concourse.tile as tile
    from concourse.masks import make_identity

    f32 = mybir.dt.float32
    ngroups = nb // 4
    assert ngroups % gpc == 0
    nchunk = ngroups // gpc
    assert gpc % SUP == 0
    spc = gpc // SUP  # supergroups per chunk
    rows = nb * H

    # Bacc.finalize() runs the legalization pipeline (event-semaphore
    # splitting for walrus's one-wait-per-instruction limit, nop fusion)
    nc = bacc.Bacc()
    q_d = nc.declare_dram_parameter("q", [rows, D], f32, isOutput=False)
    k_d = nc.declare_dram_parameter("k", [rows, D], f32, isOutput=False)
    v_d = nc.declare_dram_parameter("v", [rows, D], f32, isOutput=False)
    o_d = nc.declare_dram_parameter("out", [rows, D], f32, isOutput=True)

    # chunk views: [chunk, partition(=4b*32h within group), group, d]
    qv = q_d.rearrange("(c g p) d -> c p g d", p=128, g=gpc)
    kv = k_d.rearrange("(c g p) d -> c p g d", p=128, g=gpc)
    vv = v_d.rearrange("(c g p) d -> c p g d", p=128, g=gpc)
    ov = o_d.rearrange("(c g p) d -> c p g d", p=128, g=gpc)

    with tile.TileContext(nc) as tc:
        with (
            tc.tile_pool(name="const", bufs=1) as cpool,
            tc.tile_pool(name="chunk", bufs=3) as chpool,
            tc.tile_pool(name="work", bufs=4) as wpool,
            tc.tile_pool(name="psum", bufs=2, space="PSUM") as pspool,
        ):
            ident = cpool.tile([128, 128], f32)
            make_identity(nc, ident[:])
            # zero-output ldweights absorbs the gpsimd identity-ready wait so
            # no real matmul ever carries it (matmul's S3_LW lowering has a
            # single wait slot); the loaded weights are never used
            nc.tensor.ldweights(ident[:, 0:64].bitcast(mybir.dt.bfloat16))

            if loop_T > 1:
                loop_cm = tc.For_i(
                    0,
                    loop_T,
                    1,
                    hint_engines=(
                        mybir.EngineType.PE,
                        mybir.EngineType.Activation,
                        mybir.EngineType.DVE,
                        mybir.EngineType.SP,
                    ),
                )
            else:
                loop_cm = contextlib.nullcontext()
            with loop_cm:
              for c in range(nchunk):
                q_ch = chpool.tile([128, gpc, D], f32, tag="q_ch")
                k_ch = chpool.tile([128, gpc, D], f32, tag="k_ch")
                v_ch = chpool.tile([128, gpc, D], f32, tag="v_ch")
                o_ch = chpool.tile([128, gpc, D], f32, tag="o_ch")
                # split across both HWDGE rings (SP + ACT) to double
                # descriptor-generation throughput
                nc.sync.dma_start(q_ch[:], qv[c])
                nc.sync.dma_start(k_ch[:], kv[c])
                nc.sync.dma_start(v_ch[:], vv[c])
                # zero-output ldweights absorb each chunk-DMA wait on PE so
                # no real matmul carries a DMA wait alongside a slot-release
                # wait (matmul lowering has one wait slot)
                nc.tensor.ldweights(q_ch[0:32, 0, 0:64].bitcast(mybir.dt.bfloat16))
                nc.tensor.ldweights(k_ch[0:32, 0, 0:64].bitcast(mybir.dt.bfloat16))
                nc.tensor.ldweights(v_ch[0:32, 0, 0:64].bitcast(mybir.dt.bfloat16))

                # tiny first-accessor write: carries o_ch's slot-release wait
                # (out-DMA of chunk c-2) so the real DVE writes only wait on PE
                nc.vector.tensor_copy(o_ch[0:1, 0, 0:1], ident[0:1, 0:1])

                if "compute" in ablate:
                    nc.sync.dma_start(ov[c], q_ch[:])
                    continue

                for s in range(spc):
                    g0 = s * SUP
                    if "transpose" in ablate:
                        qt = q_ch[:, g0 : g0 + SUP, :]
                        kt = k_ch[:, g0 : g0 + SUP, :]
                    else:
                        ps_qt = pspool.tile([128, SUP, D], f32, tag="ps_qt")
                        ps_kt = pspool.tile([128, SUP, D], f32, tag="ps_kt")
                        for gi in range(SUP):
                            nc.tensor.transpose(
                                ps_qt[:, gi, :], q_ch[:, g0 + gi, :], ident[:]
                            )
                            nc.tensor.transpose(
                                ps_kt[:, gi, :], k_ch[:, g0 + gi, :], ident[:]
                            )
                        qt = wpool.tile([128, SUP, D], f32, tag="qt")
                        kt = wpool.tile([128, SUP, D], f32, tag="kt")
                        # balance PSUM->SBUF copies across ACT and DVE
                        # (bacc's event-sem legalization handles the matmul
                        # wait fan-in)
                        nc.scalar.copy(qt[:], ps_qt[:])
                        nc.vector.tensor_copy(kt[:], ps_kt[:])

                    ps_s = pspool.tile([128, SUP, 32], f32, tag="ps_s")
                    for gi in range(SUP):
                        for j in range(4):
                            nc.tensor.matmul(
                                ps_s[32 * j : 32 * j + 32, gi, :],
                                qt[:, gi, 32 * j : 32 * j + 32],
                                kt[:, gi, 32 * j : 32 * j + 32],
                                tile_position=(0, 32 * j),
                            )

                    p_t = wpool.tile([128, SUP, 32], f32, tag="p_t")
                    # first-accessor absorber: carries p_t's slot-release wait
                    # (DVE StreamTranspose of supergroup s-2)
                    nc.scalar.copy(p_t[0:1, 0, 0:1], ident[0:1, 0:1])
                    nc.scalar.activation(
                        p_t[:],
                        ps_s[:],
                        mybir.ActivationFunctionType.Exp,
                        scale=SCALE,
                    )
                    den = wpool.tile([128, SUP], f32, tag="den")
                    nc.vector.reduce_sum(den[:], p_t[:], axis=mybir.AxisListType.X)
                    rec = wpool.tile([128, SUP], f32, tag="rec")
                    nc.vector.reciprocal(rec[:], den[:])

                    pt = wpool.tile([128, SUP, 32], f32, tag="pt")
                    # first-accessor absorber: carries pt's slot-release wait
                    # (PE PV matmuls of supergroup s-2)
                    nc.vector.tensor_copy(pt[0:1, 0, 0:1], ident[0:1, 0:1])
                    nc.vector.transpose(
                        pt[:].rearrange("p g k -> p (g k)"),
                        p_t[:].rearrange("p g k -> p (g k)"),
                    )

                    ps_o = pspool.tile([128, SUP, D], f32, tag="ps_o")
                    if "pv" in ablate:
                        for gi in range(SUP):
                            nc.tensor.matmul(
                                ps_o[0:32, gi, :],
                                pt[0:32, gi, :],
                                v_ch[0:32, g0 + gi, :],
                                tile_position=(0, 0),
                            )
                    else:
                        for gi in range(SUP):
                            for j in range(4):
                                nc.tensor.matmul(
                                    ps_o[32 * j : 32 * j + 32, gi, :],
                                    pt[32 * j : 32 * j + 32, gi, :],
                                    v_ch[32 * j : 32 * j + 32, g0 + gi, :],
                                    tile_position=(32 * j, 32 * j),
                                )

                    nc.vector.tensor_tensor(
                        o_ch[:, g0 : g0 + SUP, :],
                        ps_o[:],
                        rec[:, :, None].to_broadcast([128, SUP, D]),
                        mybir.AluOpType.mult,
                    )

                nc.sync.dma_start(ov[c], o_ch[:])

    nc.finalize()
    return nc


_NC_CACHE = {}


def _get_nc(nb=NB, gpc=16):
    key = (nb, gpc)
    if key not in _NC_CACHE:
        _NC_CACHE[key] = build_kernel(nb, gpc)
    return _NC_CACHE[key]


def kernel(q, k, v, k_cache, v_cache, slot_mapping):
    """Full-input entry point: shards batch across 8 cores, returns full output."""
    from concourse.bass_utils import run_bass_kernel_spmd

    nc = _get_nc()
    q = np.ascontiguousarray(np.asarray(q, dtype=np.float32))
    k = np.ascontiguousarray(np.asarray(k, dtype=np.float32))
    v = np.ascontiguousarray(np.asarray(v, dtype=np.float32))
    in_maps = [
        {
            "q": q[i * NB : (i + 1) * NB].reshape(NB * H, D),
            "k": k[i * NB : (i + 1) * NB].reshape(NB * H, D),
            "v": v[i * NB : (i + 1) * NB].reshape(NB * H, D),
        }
        for i in range(NCORES)
    ]
    res = run_bass_kernel_spmd(nc, in_maps, list(range(NCORES))).results
    out = np.concatenate(
        [res[i]["out"].reshape(NB, H * D) for i in range(NCORES)], axis=0
    )
    return out


# revision 25
# speedup vs baseline: 1.3270x; 1.3270x over previous
"""Trainium2 Bass kernel for nn_Attention_86165633892896 (sparse_attention).

Math: the reference scatters fresh k/v rows into a paged KV cache at
collision-free slots, then immediately gathers the same slots back out.
With unique slots, gather(scatter(cache, s, x), s) == x exactly, so the
cache round-trip is an identity and the output depends only on q, k, v:

    out[b] = softmax(Q_b @ K_b^T * scale) @ V_b        (per batch b)

with Q_b, K_b, V_b of shape [32, 128]  (32 heads, head_dim 128), B = 4096.

Scores are bounded (|s| < ~6 for randn inputs), so softmax without
max-subtraction is numerically safe in fp32 and matches jax.nn.softmax to
fp32 rounding.

Mapping to one NeuronCore (data-parallel over B, 512 batches/core):
  * batches are processed in "groups" of 4 -> a [128, 128] tile whose
    partition axis is (b_local*32 + head) and free axis is head_dim d.
  * Q,K chunks are loaded FULLY CONTIGUOUSLY (partition p holds gpc
    consecutive rows -> 8KB DMA descriptors instead of 512B, ~12% less DMA
    time); the PE transposes that put d on partitions anyway also repair
    the layout: transposing q_ch[:, w, :] yields Q^T columns for rows
    {gpc*p + w}, and the PSUM->SBUF copy scatters column (w, p) to flat
    column gpc*p + w = the global row index, restoring natural order.
    V and the output keep the strided row-per-partition layout (512B
    pieces) because the PV matmul needs V rows k-ordered on partitions.
  * QK^T: 4 column-tiled matmuls (tile_position=(0,32j)), one per batch,
    stationary = Q^T[:, 32j:32j+32], moving = K^T[:, 32j:32j+32].
    Output lands compactly as PSUM [128=(4b,h), 32=k].
  * softmax: one ACT exp (scale folded in), one DVE reduce_sum, one DVE
    reciprocal.  1/denominator is folded into the output copy.
  * P^T: one DVE StreamTranspose (in-place 32x32 block transposes).
  * PV: 4 diagonal-tiled matmuls (tile_position=(32j,32j)), stationary =
    P_j^T [32k, 32h], moving = natural V rows [32k, 128d].  Output is the
    natural output layout [128=(4b,h), 128=d] in PSUM.
  * output: one DVE tensor_tensor multiply by broadcast reciprocal,
    PSUM -> SBUF, then contiguous DMA out.
Four groups form a "supergroup" sharing single softmax/copy instructions;
chunks of 16 groups share 1 MiB DMAs.
"""

import numpy as np

B = 4096
H = 32
D = 128
SCALE = 0.08838834764831845
NCORES = 8
NB = B // NCORES  # 512 batches per core

SUP = 4  # groups per supergroup (16 batches)


def build_kernel(nb=NB, gpc=16, loop_T=1, ablate=(), contig_qk=True):
    """Build the per-core Bass kernel for nb batches, gpc groups per DMA chunk.

    loop_T > 1 wraps the whole body in a For_i that repeats it (identical
    work each iteration) -- used only for device-time measurement.
    """
    import contextlib

    import concourse.bacc as bacc
    import concourse.mybir as mybir
    import concourse.tile as tile
    from concourse.masks import make_identity

    f32 = mybir.dt.float32
    ngroups = nb // 4
    assert ngroups % gpc == 0
    nchunk = ngroups // gpc
    assert gpc % SUP == 0
    spc = gpc // SUP  # supergroups per chunk
    rows = nb * H

    # Bacc.finalize() runs the legalization pipeline (event-semaphore
    # splitting for walrus's one-wait-per-instruction limit, nop fusion)
    nc = bacc.Bacc()
    q_d = nc.declare_dram_parameter("q", [rows, D], f32, isOutput=False)
    k_d = nc.declare_dram_parameter("k", [rows, D], f32, isOutput=False)
    v_d = nc.declare_dram_parameter("v", [rows, D], f32, isOutput=False)
    o_d = nc.declare_dram_parameter("out", [rows, D], f32, isOutput=True)

    # chunk views: [chunk, partition(=4b*32h within group), group, d]
    if contig_qk:
        # fully-contiguous load: partition p holds gpc consecutive rows
        # (8KB descriptors instead of 512B); the PE transposes repair the
        # layout for free and the matmul APs un-permute the column order
        assert 32 % gpc == 0
        qv = q_d.rearrange("(c p w) d -> c p (w d)", p=128, w=gpc)
        kv = k_d.rearrange("(c p w) d -> c p (w d)", p=128, w=gpc)
    else:
        qv = q_d.rearrange("(c g p) d -> c p g d", p=128, g=gpc)
        kv = k_d.rearrange("(c g p) d -> c p g d", p=128, g=gpc)
    vv = v_d.rearrange("(c g p) d -> c p g d", p=128, g=gpc)
    ov = o_d.rearrange("(c g p) d -> c p g d", p=128, g=gpc)

    with tile.TileContext(nc) as tc:
        with (
            tc.tile_pool(name="const", bufs=1) as cpool,
            tc.tile_pool(name="chunk", bufs=3) as chpool,
            tc.tile_pool(name="work", bufs=4) as wpool,
            tc.tile_pool(name="psum", bufs=2, space="PSUM") as pspool,
        ):
            ident = cpool.tile([128, 128], f32)
            make_identity(nc, ident[:])
            # zero-output ldweights absorbs the gpsimd identity-ready wait so
            # no real matmul ever carries it (matmul's S3_LW lowering has a
            # single wait slot); the loaded weights are never used
            nc.tensor.ldweights(ident[:, 0:64].bitcast(mybir.dt.bfloat16))

            if loop_T > 1:
                loop_cm = tc.For_i(
                    0,
                    loop_T,
                    1,
                    hint_engines=(
                        mybir.EngineType.PE,
                        mybir.EngineType.Activation,
                        mybir.EngineType.DVE,
                        mybir.EngineType.SP,
                    ),
                )
            else:
                loop_cm = contextlib.nullcontext()
            with loop_cm:
              for c in range(nchunk):
                q_ch = chpool.tile([128, gpc, D], f32, tag="q_ch")
                k_ch = chpool.tile([128, gpc, D], f32, tag="k_ch")
                v_ch = chpool.tile([128, gpc, D], f32, tag="v_ch")
                o_ch = chpool.tile([128, gpc, D], f32, tag="o_ch")
                # split across both HWDGE rings (SP + ACT) to double
                # descriptor-generation throughput
                nc.sync.dma_start(q_ch[:], qv[c])
                nc.sync.dma_start(k_ch[:], kv[c])
                nc.sync.dma_start(v_ch[:], vv[c])
                # zero-output ldweights absorb each chunk-DMA wait on PE so
                # no real matmul carries a DMA wait alongside a slot-release
                # wait (matmul lowering has one wait slot)
                nc.tensor.ldweights(q_ch[0:32, 0, 0:64].bitcast(mybir.dt.bfloat16))
                nc.tensor.ldweights(k_ch[0:32, 0, 0:64].bitcast(mybir.dt.bfloat16))
                nc.tensor.ldweights(v_ch[0:32, 0, 0:64].bitcast(mybir.dt.bfloat16))

                # tiny first-accessor write: carries o_ch's slot-release wait
                # (out-DMA of chunk c-2) so the real DVE writes only wait on PE
                nc.vector.tensor_copy(o_ch[0:1, 0, 0:1], ident[0:1, 0:1])

                if "compute" in ablate:
                    nc.sync.dma_start(ov[c], q_ch[:])
                    continue

                if contig_qk:
                    # gpc w-transposes per tensor put d on partitions for the
                    # whole chunk; the PSUM->SBUF copy scatters transpose
                    # column (w, p) to flat column gpc*p + w = global row, so
                    # qt_sb[d, R] is Q^T in natural row order and matmul
                    # operand slices are contiguous single-free-dim APs
                    qt_sb = chpool.tile([128, 128, gpc], f32, tag="qt_sb")
                    kt_sb = chpool.tile([128, 128, gpc], f32, tag="kt_sb")
                    qt_w = qt_sb[:].rearrange("a p w -> a w p")
                    kt_w = kt_sb[:].rearrange("a p w -> a w p")
                    for q4 in range(gpc // SUP):
                        ps_qt = pspool.tile([128, SUP, 128], f32, tag="ps_qt")
                        ps_kt = pspool.tile([128, SUP, 128], f32, tag="ps_kt")
                        for wi in range(SUP):
                            w = q4 * SUP + wi
                            nc.tensor.transpose(
                                ps_qt[:, wi, :], q_ch[:, w, :], ident[:]
                            )
                            nc.tensor.transpose(
                                ps_kt[:, wi, :], k_ch[:, w, :], ident[:]
                            )
                        nc.scalar.copy(
                            qt_w[:, q4 * SUP : (q4 + 1) * SUP, :], ps_qt[:]
                        )
                        nc.vector.tensor_copy(
                            kt_w[:, q4 * SUP : (q4 + 1) * SUP, :], ps_kt[:]
                        )
                    qt_f = qt_sb[:].rearrange("a p w -> a (p w)")
                    kt_f = kt_sb[:].rearrange("a p w -> a (p w)")

                for s in range(spc):
                    g0 = s * SUP
                    if contig_qk:
                        pass
                    elif "transpose" in ablate:
                        qt = q_ch[:, g0 : g0 + SUP, :]
                        kt = k_ch[:, g0 : g0 + SUP, :]
                    else:
                        ps_qt = pspool.tile([128, SUP, D], f32, tag="ps_qt")
                        ps_kt = pspool.tile([128, SUP, D], f32, tag="ps_kt")
                        for gi in range(SUP):
                            nc.tensor.transpose(
                                ps_qt[:, gi, :], q_ch[:, g0 + gi, :], ident[:]
                            )
                            nc.tensor.transpose(
                                ps_kt[:, gi, :], k_ch[:, g0 + gi, :], ident[:]
                            )
                        qt = wpool.tile([128, SUP, D], f32, tag="qt")
                        kt = wpool.tile([128, SUP, D], f32, tag="kt")
                        # balance PSUM->SBUF copies across ACT and DVE
                        # (bacc's event-sem legalization handles the matmul
                        # wait fan-in)
                        nc.scalar.copy(qt[:], ps_qt[:])
                        nc.vector.tensor_copy(kt[:], ps_kt[:])

                    ps_s = pspool.tile([128, SUP, 32], f32, tag="ps_s")
                    for gi in range(SUP):
                        for j in range(4):
                            if contig_qk:
                                bch = (g0 + gi) * 4 + j  # batch index in chunk
                                lhsT = qt_f[:, 32 * bch : 32 * bch + 32]
                                rhs = kt_f[:, 32 * bch : 32 * bch + 32]
                            else:
                                lhsT = qt[:, gi, 32 * j : 32 * j + 32]
                                rhs = kt[:, gi, 32 * j : 32 * j + 32]
                            nc.tensor.matmul(
                                ps_s[32 * j : 32 * j + 32, gi, :],
                                lhsT,
                                rhs,
                                tile_position=(0, 32 * j),
                            )

                    p_t = wpool.tile([128, SUP, 32], f32, tag="p_t")
                    # first-accessor absorber: carries p_t's slot-release wait
                    # (DVE StreamTranspose of supergroup s-2)
                    nc.scalar.copy(p_t[0:1, 0, 0:1], ident[0:1, 0:1])
                    nc.scalar.activation(
                        p_t[:],
                        ps_s[:],
                        mybir.ActivationFunctionType.Exp,
                        scale=SCALE,
                    )
                    den = wpool.tile([128, SUP], f32, tag="den")
                    nc.vector.reduce_sum(den[:], p_t[:], axis=mybir.AxisListType.X)
                    rec = wpool.tile([128, SUP], f32, tag="rec")
                    nc.vector.reciprocal(rec[:], den[:])

                    pt = wpool.tile([128, SUP, 32], f32, tag="pt")
                    # first-accessor absorber: carries pt's slot-release wait
                    # (PE PV matmuls of supergroup s-2)
                    nc.vector.tensor_copy(pt[0:1, 0, 0:1], ident[0:1, 0:1])
                    nc.vector.transpose(
                        pt[:].rearrange("p g k -> p (g k)"),
                        p_t[:].rearrange("p g k -> p (g k)"),
                    )

                    ps_o = pspool.tile([128, SUP, D], f32, tag="ps_o")
                    if "pv" in ablate:
                        for gi in range(SUP):
                            nc.tensor.matmul(
                                ps_o[0:32, gi, :],
                                pt[0:32, gi, :],
                                v_ch[0:32, g0 + gi, :],
                                tile_position=(0, 0),
                            )
                    else:
                        for gi in range(SUP):
                            for j in range(4):
                                nc.tensor.matmul(
                                    ps_o[32 * j : 32 * j + 32, gi, :],
                                    pt[32 * j : 32 * j + 32, gi, :],
                                    v_ch[32 * j : 32 * j + 32, g0 + gi, :],
                                    tile_position=(32 * j, 32 * j),
                                )

                    nc.vector.tensor_tensor(
                        o_ch[:, g0 : g0 + SUP, :],
                        ps_o[:],
                        rec[:, :, None].to_broadcast([128, SUP, D]),
                        mybir.AluOpType.mult,
                    )

                nc.sync.dma_start(ov[c], o_ch[:])

    nc.finalize()
    return nc


_NC_CACHE = {}


def _get_nc(nb=NB, gpc=16):
    key = (nb, gpc)
    if key not in _NC_CACHE:
        _NC_CACHE[key] = build_kernel(nb, gpc)
    return _NC_CACHE[key]


def kernel(q, k, v, k_cache, v_cache, slot_mapping):
    """Full-input entry point: shards batch across 8 cores, returns full output."""
    from concourse.bass_utils import run_bass_kernel_spmd

    nc = _get_nc()
    q = np.ascontiguousarray(np.asarray(q, dtype=np.float32))
    k = np.ascontiguousarray(np.asarray(k, dtype=np.float32))
    v = np.ascontiguousarray(np.asarray(v, dtype=np.float32))
    in_maps = [
        {
            "q": q[i * NB : (i + 1) * NB].reshape(NB * H, D),
            "k": k[i * NB : (i + 1) * NB].reshape(NB * H, D),
            "v": v[i * NB : (i + 1) * NB].reshape(NB * H, D),
        }
        for i in range(NCORES)
    ]
    res = run_bass_kernel_spmd(nc, in_maps, list(range(NCORES))).results
    out = np.concatenate(
        [res[i]["out"].reshape(NB, H * D) for i in range(NCORES)], axis=0
    )
    return out
